# revision 19
# baseline (speedup 1.0000x reference)
"""Cached self-attention Trainium2 kernel (v4).

Sharding: 8 cores = 2 batches x 4 head-groups. Core c: batch b=c//4, group
g=c%4 owns heads 4g..4g+3 (columns 512g:512g+512 of the q/k/v projections).
Each core projects q/k/v for its heads over the full sequence, runs attention
for its 4 heads, the 4 cores of a batch AllGather the (normalized, transposed)
per-head attention outputs, and each core computes the output projection onto
its 512-column slice of wo (full sequence), so outputs tile the model dim.

v4 restructure (vs v3): per-head software pipeline. Head j+1's q/k projection
matmuls and the out-projection partial sums act as TensorE filler while head
j's attention is paced by the exp (ScalarE) stream, so the PE never idles long
enough to trip the HAM idle-throttle. All inputs are pre-packed on the host so
every DMA is partition-contiguous. The softmax epilogue is moved off ScalarE:
the 1/sqrt(HD) scale is folded into the exp activation, q/k bias adds are DVE
tensor_scalar ops, v/out biases are broadcast tiles added during PSUM
evacuation on DVE. PSUM: 2 rotating 2-bank score groups (exp-paced), PA
double-buffered, 2 projection banks (reused by out-proj chains). Out-proj is
split per-head: heads 0-2 accumulate into an SBUF partial while head 3's
attention still runs; only the final 4-matmul chain waits on the last gather.
"""
import numpy as np
from contextlib import ExitStack

import concourse.bass as bass
import concourse.tile as tile
from concourse import bacc, mybir
from concourse.bass_utils import run_bass_kernel_spmd

B, S, PC, D, H = 2, 2048, 2048, 2048, 16
HD = D // H            # 128 head dim
GH = H // 4            # 4 heads per core
DG = GH * HD           # 512 head-dims per core
NB = 512               # block size
NKC = (PC + S) // HD   # 32 key chunks of 128
NCC = PC // HD         # 16 cached key chunks
NDC = D // HD          # 16 contraction chunks
NSS = S // HD          # 16 new-key chunks
F16 = mybir.dt.float16
F32 = mybir.dt.float32
AF = mybir.ActivationFunctionType
ALU = mybir.AluOpType
INV_SQRT_HD = float(1.0 / np.sqrt(HD))

GROUPS = [[0, 1, 2, 3], [4, 5, 6, 7]]


def build():
    nc = bacc.Bacc("TRN2", target_bir_lowering=False, debug=False, num_devices=8)

    def inp(name, shape, dt=F16):
        return nc.dram_tensor(name, shape, dt, kind="ExternalInput").ap()

    # all host-side pre-packed for contiguous per-partition DMA
    xp = inp("xp", [HD, NDC, S])          # xp[p,kc,s] = x[b][s, kc*128+p]
    wqp = inp("wqp", [HD, GH, NDC, HD])   # [p,j,kc,n] = wq[kc*128+p, 512g+128j+n]
    wkp = inp("wkp", [HD, GH, NDC, HD])
    wvp = inp("wvp", [HD, NDC, DG])       # [p,kc,n] = wv[kc*128+p, sl_n]
    bqp = inp("bqp", [HD, GH], F32)       # [p,j] = bq[sl][128j+p]
    bkp = inp("bkp", [HD, GH], F32)
    bvp = inp("bvp", [1, DG])
    bop = inp("bop", [1, DG])
    ckp = inp("ckp", [HD, GH, PC])        # [p,j,key] = cache_k[b][key, 512g+128j+p]
    cvp = inp("cvp", [HD, NCC, DG])       # [p,ss,d] = cache_v[b][ss*128+p, sl_d]
    wop = inp("wop", [HD, 16, DG])        # rows permuted to gather order, packed
    y = nc.dram_tensor("y", [S, DG], F32, kind="ExternalOutput").ap()

    with tile.TileContext(nc) as tc, ExitStack() as ctx:
        res = ctx.enter_context(tc.tile_pool(name="res", bufs=1))
        dram = ctx.enter_context(tc.tile_pool(name="dram", bufs=1, space="DRAM"))
        ps = ctx.enter_context(tc.tile_pool(name="ps", bufs=1, space="PSUM"))

        # whole-kernel residents
        bq_t = res.tile([HD, GH], F32, tag="bq")
        bk_t = res.tile([HD, GH], F32, tag="bk")
        bv_t = res.tile([1, DG], F16, tag="bv")
        bo_t = res.tile([1, DG], F16, tag="bo")
        ones_k = res.tile([HD, 1], F16, tag="ones_k")      # [128,1] ones
        ones_r16 = res.tile([1, HD], F16, tag="ones_r16")  # [1,128] ones
        ones_r32 = res.tile([1, HD], F32, tag="ones_r32")
        nc.sync.dma_start(bq_t[:], bqp)
        nc.sync.dma_start(bk_t[:], bkp)
        nc.sync.dma_start(bv_t[:], bvp)
        nc.sync.dma_start(bo_t[:], bop)
        nc.vector.memset(ones_k[:], 1.0)
        nc.vector.memset(ones_r16[:], 1.0)
        nc.vector.memset(ones_r32[:], 1.0)

        # broadcast bias tiles (bias varies along the free dim, so build
        # [128, 512] broadcast copies once via ones-matmuls)
        bv_bc = res.tile([HD, DG], F16, tag="bv_bc")
        bo_bc = res.tile([HD, DG], F32, tag="bo_bc")
        psx = ps.tile([HD, DG], F32, tag="pq0", name="ps_bv")
        nc.tensor.matmul(psx[:], ones_r16[:], bv_t[:], start=True, stop=True)
        nc.vector.tensor_copy(bv_bc[:], psx[:])
        psx = ps.tile([HD, DG], F32, tag="PA0", name="ps_bo")
        nc.tensor.matmul(psx[:], ones_r16[:], bo_t[:], start=True, stop=True)
        nc.vector.tensor_copy(bo_bc[:], psx[:])

        # collective bounce buffers: heads 0-2 gather per sb-pair half so
        # each half-gather fires as soon as two sb blocks are normalized;
        # head 3 gathers per sb quarter to minimize the end-of-kernel wait
        bounce_in = {}
        bounce_out = {}
        for j in range(3):
            for h in range(2):
                bounce_in[j, h] = dram.tile([HD, 2, NB], F16,
                                            tag=f"bi{j}_{h}", name=f"bi{j}_{h}")
                bounce_out[j, h] = dram.tile([4, HD, 2, NB], F16,
                                             tag=f"bg{j}_{h}",
                                             name=f"bg{j}_{h}")
        for qq in range(4):
            bounce_in[3, qq] = dram.tile([HD, 1, NB], F16,
                                         tag=f"bi3_{qq}", name=f"bi3_{qq}")
            bounce_out[3, qq] = dram.tile([4, HD, 1, NB], F16,
                                          tag=f"bg3_{qq}", name=f"bg3_{qq}")

        # long-lived attention pools (heads 0..3)
        hp = ctx.enter_context(tc.tile_pool(name="hp", bufs=2))
        vp = ctx.enter_context(tc.tile_pool(name="vp", bufs=1))
        ep = ctx.enter_context(tc.tile_pool(name="ep", bufs=8))
        zp = ctx.enter_context(tc.tile_pool(name="zp", bufs=2))
        apool = ctx.enter_context(tc.tile_pool(name="ap", bufs=2))

        cv_t = vp.tile([HD, NCC, DG], F16, tag="cv")
        vn_t = vp.tile([HD, NSS, DG], F16, tag="vn")

        pq_cnt = [0]
        PRO_TAGS = ["pq0", "PA0", "pss0", "pss1", "pss2"]

        def proj_chunk(wt, kc_tiles, dst_ap, bias_ap, tags=("pq0",)):
            """One [128, 512] projection chunk: 16 accumulating matmuls +
            DVE evacuation with per-partition bias add."""
            psq = ps.tile([HD, NB], F32, tag=tags[pq_cnt[0] % len(tags)],
                          name="psq")
            pq_cnt[0] += 1
            for kc in range(NDC):
                nc.tensor.matmul(psq[:], wt[:, kc, :], kc_tiles[kc],
                                 start=(kc == 0), stop=(kc == NDC - 1))
            nc.vector.tensor_scalar(dst_ap, psq[:], bias_ap, None, ALU.add)

        def v_chunk(ss, xg):
            psv = ps.tile([HD, DG], F32,
                          tag=PRO_TAGS[pq_cnt[0] % len(PRO_TAGS)], name="psv")
            pq_cnt[0] += 1
            for kc in range(NDC):
                nc.tensor.matmul(psv[:],
                                 xg[kc // 4][:, kc % 4, HD * ss:HD * (ss + 1)],
                                 wvt[:, kc, :],
                                 start=(kc == 0), stop=(kc == NDC - 1))
            # vn = psv + bv (bias varies along free dim -> broadcast add)
            nc.vector.scalar_tensor_tensor(vn_t[:, ss, :], psv[:], 1.0,
                                           bv_bc[:], ALU.mult, ALU.add)

        def attention(j, qT, kT, ckT, quarters=False):
            nah = 4 if quarters else 2
            aheads = [apool.tile([HD, 2 // (nah // 2), NB], F16, tag=f"ah{h}",
                                 name=f"ah{j}_{h}")
                      for h in range(nah)]
            for sb in range(4):
                PA = ps.tile([HD, NB], F32, tag="PA0", name="PA")
                zacc = zp.tile([HD, 2, NB], F16, tag="z")
                qs = qT[:, NB * sb:NB * (sb + 1)]
                for c2 in range(NKC // 2):
                    pss = ps.tile([HD, 2, NB], F32, tag=f"pss{c2 % 3}",
                                  name="pss")
                    e2 = ep.tile([HD, 2, NB], F16, tag="e")
                    for i in range(2):
                        c = 2 * c2 + i
                        if c < NCC:
                            kt = ckT[:, HD * c:HD * (c + 1)]
                        else:
                            kt = kT[:, HD * (c - NCC):HD * (c - NCC + 1)]
                        nc.tensor.matmul(pss[:, i, :], kt, qs,
                                         start=True, stop=True)
                    nc.scalar.activation(e2[:], pss[:], AF.Exp,
                                         scale=INV_SQRT_HD)
                    if c2 == 0:
                        nc.vector.tensor_copy(zacc[:], e2[:])
                    else:
                        nc.vector.tensor_tensor(zacc[:], zacc[:], e2[:],
                                                ALU.add)
                    for i in range(2):
                        c = 2 * c2 + i
                        if c < NCC:
                            vt = cv_t[:, c, HD * j:HD * (j + 1)]
                        else:
                            vt = vn_t[:, c - NCC, HD * j:HD * (j + 1)]
                        nc.tensor.matmul(PA[:], vt, e2[:, i, :],
                                         start=(c == 0), stop=(c == NKC - 1),
                                         skip_group_check=True)
                # softmax denominator: accumulate the two zacc halves via two
                # ones-matmuls, fast reciprocal, broadcast, normalize PA
                psz = ps.tile([1, NB], F32, tag="pss1", name="psz")
                nc.tensor.matmul(psz[:], ones_k[:], zacc[:, 0, :],
                                 start=True, stop=False)
                nc.tensor.matmul(psz[:], ones_k[:], zacc[:, 1, :],
                                 start=False, stop=True)
                zinv = zp.tile([1, NB], F32, tag="zi")
                nc.vector.reciprocal_approx_fast(zinv[:], psz[:])
                zinv16 = zp.tile([1, NB], F16, tag="zi16")
                nc.vector.tensor_copy(zinv16[:], zinv[:])
                psb = ps.tile([HD, NB], F32, tag="pss2", name="psb")
                nc.tensor.matmul(psb[:], ones_r16[:], zinv16[:],
                                 start=True, stop=True)
                zb = zp.tile([HD, NB], F32, tag="zb")
                nc.vector.tensor_copy(zb[:], psb[:])
                if quarters:
                    nc.vector.tensor_tensor(aheads[sb][:, 0, :], PA[:],
                                            zb[:], ALU.mult)
                    nc.sync.dma_start(bounce_in[j, sb][:], aheads[sb][:])
                    nc.gpsimd.collective_compute(
                        "AllGather", ALU.bypass, replica_groups=GROUPS,
                        ins=[bounce_in[j, sb].opt()],
                        outs=[bounce_out[j, sb].opt()])
                else:
                    nc.vector.tensor_tensor(aheads[sb // 2][:, sb % 2, :],
                                            PA[:], zb[:], ALU.mult)
                    if sb % 2 == 1:
                        h = sb // 2
                        nc.sync.dma_start(bounce_in[j, h][:], aheads[h][:])
                        nc.gpsimd.collective_compute(
                            "AllGather", ALU.bypass, replica_groups=GROUPS,
                            ins=[bounce_in[j, h].opt()],
                            outs=[bounce_out[j, h].opt()])

        with ExitStack() as xw:
            xpool = xw.enter_context(tc.tile_pool(name="xp", bufs=1))
            wpool = xw.enter_context(tc.tile_pool(name="wp", bufs=2))
            vwpool = xw.enter_context(tc.tile_pool(name="vw", bufs=1))

            # weights for head 0 first (q0's first matmul needs them), then x
            # as a serialized stream so the first groups land ASAP; the bulk
            # loads (wv, cache) are forced behind x so they don't steal HBM
            # bandwidth from the critical startup path
            wq_t = {0: wpool.tile([HD, NDC, HD], F16, tag="wq", name="wq0")}
            wk_t = {0: wpool.tile([HD, NDC, HD], F16, tag="wk", name="wk0")}
            nc.sync.dma_start(wq_t[0][:], wqp[:, 0])
            nc.sync.dma_start(wk_t[0][:], wkp[:, 0])
            xg = []
            last_xdma = None
            for i in range(4):
                t = xpool.tile([HD, 4, S], F16, tag=f"xg{i}", name=f"xg{i}")
                # group 0 on the HW queues alone (full bandwidth); later
                # groups chained via gpsimd software DGE so they land in
                # consumption order instead of racing each other
                if i == 0:
                    last_xdma = nc.sync.dma_start(t[:],
                                                  xp[:, 4 * i:4 * (i + 1), :])
                else:
                    di = nc.gpsimd.dma_start(t[:], xp[:, 4 * i:4 * (i + 1), :])
                    tile.add_dep_helper(di.ins, last_xdma.ins,
                                        reason="serialize x stream")
                    last_xdma = di
                xg.append(t)
            # wv/cache-v are not needed until the v pass / PV chains; issue
            # them from the gpsimd software DGE gated on the x stream so the
            # HW queues give x the full HBM bandwidth at startup
            wvt = vwpool.tile([HD, NDC, DG], F16, tag="wvt")
            for dst, src in ((wvt[:], wvp), (cv_t[:], cvp)):
                di = nc.gpsimd.dma_start(dst, src)
                tile.add_dep_helper(di.ins, last_xdma.ins,
                                    reason="bulk loads after x stream")
            ck_t = {0: hp.tile([HD, PC], F16, tag="ckT", name="ck0")}
            di = nc.gpsimd.dma_start(ck_t[0][:], ckp[:, 0, :])
            tile.add_dep_helper(di.ins, last_xdma.ins,
                                reason="cache-k after x stream")

            def proj_head(j):
                qT = hp.tile([HD, S], F16, tag="qT", name=f"qT{j}")
                kT = hp.tile([HD, S], F16, tag="kT", name=f"kT{j}")
                for sb in range(4):
                    kcs = [xg[kc // 4][:, kc % 4, NB * sb:NB * (sb + 1)]
                           for kc in range(NDC)]
                    proj_chunk(wq_t[j][:], kcs, qT[:, NB * sb:NB * (sb + 1)],
                               bq_t[:, j:j + 1])
                    proj_chunk(wk_t[j][:], kcs, kT[:, NB * sb:NB * (sb + 1)],
                               bk_t[:, j:j + 1])
                return qT, kT

            def proj_head0():
                """Head-0 q/k with kc-group-major order over 6 concurrent
                PSUM chains, so matmuls track the streaming x DMA instead of
                stalling a single chain on each x group."""
                qT = hp.tile([HD, S], F16, tag="qT", name="qT0")
                kT = hp.tile([HD, S], F16, tag="kT", name="kT0")
                chains = [("q", sb) for sb in range(4)] + [("k", 0)]
                psqs = [ps.tile([HD, NB], F32, tag=PRO_TAGS[ci],
                                name=f"ps0_{ci}")
                        for ci in range(len(chains))]
                for grp in range(4):
                    for ci, (w, sb) in enumerate(chains):
                        wt = wq_t[0] if w == "q" else wk_t[0]
                        for kc in range(4 * grp, 4 * grp + 4):
                            nc.tensor.matmul(
                                psqs[ci][:], wt[:, kc, :],
                                xg[grp][:, kc % 4, NB * sb:NB * (sb + 1)],
                                start=(kc == 0), stop=(kc == NDC - 1))
                for ci, (w, sb) in enumerate(chains):
                    dst = qT if w == "q" else kT
                    bias = bq_t[:, 0:1] if w == "q" else bk_t[:, 0:1]
                    nc.vector.tensor_scalar(dst[:, NB * sb:NB * (sb + 1)],
                                            psqs[ci][:], bias, None, ALU.add)
                for sb in (1, 2, 3):
                    kcs = [xg[kc // 4][:, kc % 4, NB * sb:NB * (sb + 1)]
                           for kc in range(NDC)]
                    proj_chunk(wk_t[0][:], kcs, kT[:, NB * sb:NB * (sb + 1)],
                               bk_t[:, 0:1], tags=PRO_TAGS)
                return qT, kT

            def prefetch_head(jn):
                wq_t[jn] = wpool.tile([HD, NDC, HD], F16, tag="wq",
                                      name=f"wq{jn}")
                wk_t[jn] = wpool.tile([HD, NDC, HD], F16, tag="wk",
                                      name=f"wk{jn}")
                nc.sync.dma_start(wq_t[jn][:], wqp[:, jn])
                nc.sync.dma_start(wk_t[jn][:], wkp[:, jn])
                ck_t[jn] = hp.tile([HD, PC], F16, tag="ckT", name=f"ck{jn}")
                nc.sync.dma_start(ck_t[jn][:], ckp[:, jn, :])

            # head 0 projections, then the full v pass
            with nc.named_scope("proj0"):
                qkT = {0: proj_head0()}
                prefetch_head(1)
            with nc.named_scope("vpass"):
                for ss in range(NSS):
                    v_chunk(ss, xg)

            # heads 0-1: attention + later heads' projections as PE filler;
            # head-3's projection runs in head 1 so x/weight space frees
            # early enough for the out-proj tiles to load during head 2
            with nc.named_scope("head0"):
                attention(0, qkT[0][0], qkT[0][1], ck_t[0][:])
                qkT[1] = proj_head(1)
                prefetch_head(2)
            with nc.named_scope("head1"):
                attention(1, qkT[1][0], qkT[1][1], ck_t[1][:])
                qkT[2] = proj_head(2)
                prefetch_head(3)
                qkT[3] = proj_head(3)

        # phase 3 pools (reuse the x/weight SBUF space released above)
        with tc.tile_pool(name="wo3", bufs=1) as wop_pool, \
             tc.tile_pool(name="lt3", bufs=1) as ltp, \
             tc.tile_pool(name="y12", bufs=1) as y12p, \
             tc.tile_pool(name="yo", bufs=3) as yop:
            wot = wop_pool.tile([HD, 16, NB], F16, tag="wo")
            nc.sync.dma_start(wot[:], wop)
            y12 = y12p.tile([HD, NSS, NB], F16, tag="y12")
            lts = {}

            def load_lts(j):
                nh = 4 if j == 3 else 2
                for h in range(nh):
                    for r in range(4):
                        jr = 4 * j + r
                        lt = ltp.tile([HD, 2 // (nh // 2), NB], F16,
                                      tag=f"lt{jr}_{h}", name=f"lt{jr}_{h}")
                        nc.sync.dma_start(lt[:], bounce_out[j, h][r])
                        lts[jr, h] = lt

            oc = [0]

            def oproj_chunk(m, js, mode):
                """One [128, 512] out-proj chunk accumulating the listed
                heads' 4-peer contributions. Filler passes ('first'/'acc')
                stay on pq0 so they are usable as attention filler; the
                final pass alternates pq0/PA0 for a dense tail."""
                if mode == "final":
                    tag = ("pq0", "PA0")[oc[0] % 2]
                    oc[0] += 1
                else:
                    tag = "pq0"
                psO = ps.tile([HD, NB], F32, tag=tag, name=f"psO{js[0]}")
                mms = [(j, 4 * j + r) for j in js for r in range(4)]
                for n, (j, jr) in enumerate(mms):
                    if j == 3:
                        lt = lts[jr, m // 4][:, 0, HD * (m % 4):HD * (m % 4 + 1)]
                    else:
                        lt = lts[jr, m // 8][:, (m // 4) % 2,
                                             HD * (m % 4):HD * (m % 4 + 1)]
                    nc.tensor.matmul(
                        psO[:], lt, wot[:, jr, :],
                        start=(n == 0), stop=(n == len(mms) - 1),
                        skip_group_check=True)
                if mode == "first":
                    nc.vector.scalar_tensor_tensor(
                        y12[:, m, :], psO[:], 1.0, bo_bc[:],
                        ALU.mult, ALU.add)
                elif mode == "acc":
                    nc.vector.tensor_tensor(y12[:, m, :], y12[:, m, :],
                                            psO[:], ALU.add)
                else:
                    ot = yop.tile([HD, NB], F32, tag="ot")
                    nc.vector.tensor_tensor(ot[:], psO[:], y12[:, m, :],
                                            ALU.add)
                    nc.sync.dma_start(y[HD * m:HD * (m + 1), :], ot[:])

            load_lts(0)
            load_lts(1)
            # head 2: attention with head-0's out-proj partial as PE filler
            with nc.named_scope("head2"):
                attention(2, qkT[2][0], qkT[2][1], ck_t[2][:])
            with nc.named_scope("oproj0"):
                for m in range(NSS):
                    oproj_chunk(m, [0], "first")
            with nc.named_scope("oproj1"):
                for m in range(NSS):
                    oproj_chunk(m, [1], "acc")
            load_lts(2)
            # head 3: attention with out-proj leftovers + the quarter-gated
            # final chunks as PE filler
            with nc.named_scope("head3"):
                attention(3, qkT[3][0], qkT[3][1], ck_t[3][:], quarters=True)
            load_lts(3)
            # final: heads-2/3 contributions + combine + store; the first
            # half of the sequence only needs the first half-gathers
            with nc.named_scope("oproj3"):
                for m in range(NSS):
                    oproj_chunk(m, [2, 3], "final")

    nc.compile()
    return nc


_BUILT = None


def get_built():
    global _BUILT
    if _BUILT is None:
        _BUILT = build()
    return _BUILT


def _pack_kc(a):
    """[D, N] -> [128, D//128, N] with [p, kc, n] = a[kc*128+p, n]."""
    d, n = a.shape
    return np.ascontiguousarray(a.reshape(d // HD, HD, n).transpose(1, 0, 2))


def make_in_maps(x, cache_k, cache_v, wq, bq, wk, bk, wv, bv, wo, bo):
    x = np.asarray(x, np.float32)
    cache_k = np.asarray(cache_k, np.float32)
    cache_v = np.asarray(cache_v, np.float32)
    wq, bq = np.asarray(wq, np.float32), np.asarray(bq, np.float32)
    wk, bk = np.asarray(wk, np.float32), np.asarray(bk, np.float32)
    wv, bv = np.asarray(wv, np.float32), np.asarray(bv, np.float32)
    wo, bo = np.asarray(wo, np.float32), np.asarray(bo, np.float32)

    # permute wo rows to match gather order: chunk jr=(4j+r) holds head 4r+j
    perm = np.concatenate([
        np.arange(HD * (4 * r + j), HD * (4 * r + j) + HD)
        for j in range(GH) for r in range(4)
    ])
    wo_p = wo[perm, :]

    # per-batch packed x: [128, 16, S]
    xp_b = [_pack_kc(np.ascontiguousarray(x[b].T)).astype(np.float16)
            for b in range(B)]
    # per-batch packed cache_v rows: [128, 16, D] then slice cols per core
    cv_b = [np.ascontiguousarray(
        cache_v[b].reshape(NCC, HD, D).transpose(1, 0, 2)) for b in range(B)]

    in_maps = []
    for c in range(8):
        b, g = divmod(c, 4)
        sl = slice(DG * g, DG * (g + 1))
        wq_s, wk_s = wq[:, sl], wk[:, sl]
        # [p, j, kc, n]
        wqp = np.ascontiguousarray(
            wq_s.reshape(NDC, HD, GH, HD).transpose(1, 2, 0, 3)
        ).astype(np.float16)
        wkp = np.ascontiguousarray(
            wk_s.reshape(NDC, HD, GH, HD).transpose(1, 2, 0, 3)
        ).astype(np.float16)
        # ckp [p, j, key] = cache_k[b][key, 512g+128j+p]
        ck_s = cache_k[b][:, sl]                      # [PC, 512]
        ckp = np.ascontiguousarray(
            ck_s.reshape(PC, GH, HD).transpose(2, 1, 0)).astype(np.float16)
        in_maps.append({
            "xp": xp_b[b],
            "wqp": wqp,
            "wkp": wkp,
            "wvp": _pack_kc(wv[:, sl]).astype(np.float16),
            "bqp": np.ascontiguousarray(
                bq[sl].reshape(GH, HD).T).astype(np.float32),
            "bkp": np.ascontiguousarray(
                bk[sl].reshape(GH, HD).T).astype(np.float32),
            "bvp": bv[sl][None, :].astype(np.float16),
            "bop": bo[sl][None, :].astype(np.float16),
            "ckp": ckp,
            "cvp": np.ascontiguousarray(cv_b[b][:, :, sl]).astype(np.float16),
            "wop": _pack_kc(wo_p[:, sl]).astype(np.float16),
        })
    return in_maps


def assemble(results):
    out = np.empty((B, S, D), np.float32)
    for c in range(8):
        b, g = divmod(c, 4)
        out[b, :, DG * g:DG * (g + 1)] = results[c]["y"]
    return out


def kernel(**inputs):
    nc = get_built()
    in_maps = make_in_maps(**inputs)
    res = run_bass_kernel_spmd(nc, in_maps, core_ids=list(range(8)))
    return assemble(res.results)


# revision 20
# speedup vs baseline: 1.1134x; 1.1134x over previous
"""Cached self-attention Trainium2 kernel (v4).

Sharding: 8 cores = 2 batches x 4 head-groups. Core c: batch b=c//4, group
g=c%4 owns heads 4g..4g+3 (columns 512g:512g+512 of the q/k/v projections).
Each core projects q/k/v for its heads over the full sequence, runs attention
for its 4 heads, the 4 cores of a batch AllGather the (normalized, transposed)
per-head attention outputs, and each core computes the output projection onto
its 512-column slice of wo (full sequence), so outputs tile the model dim.

v4 restructure (vs v3): per-head software pipeline. Head j+1's q/k projection
matmuls and the out-projection partial sums act as TensorE filler while head
j's attention is paced by the exp (ScalarE) stream, so the PE never idles long
enough to trip the HAM idle-throttle. All inputs are pre-packed on the host so
every DMA is partition-contiguous. The softmax epilogue is moved off ScalarE:
the 1/sqrt(HD) scale is folded into the exp activation, q/k bias adds are DVE
tensor_scalar ops, v/out biases are broadcast tiles added during PSUM
evacuation on DVE. PSUM: 2 rotating 2-bank score groups (exp-paced), PA
double-buffered, 2 projection banks (reused by out-proj chains). Out-proj is
split per-head: heads 0-2 accumulate into an SBUF partial while head 3's
attention still runs; only the final 4-matmul chain waits on the last gather.
"""
import numpy as np
from contextlib import ExitStack

import concourse.bass as bass
import concourse.tile as tile
from concourse import bacc, mybir
from concourse.bass_utils import run_bass_kernel_spmd

B, S, PC, D, H = 2, 2048, 2048, 2048, 16
HD = D // H            # 128 head dim
GH = H // 4            # 4 heads per core
DG = GH * HD           # 512 head-dims per core
NB = 512               # block size
NKC = (PC + S) // HD   # 32 key chunks of 128
NCC = PC // HD         # 16 cached key chunks
NDC = D // HD          # 16 contraction chunks
NSS = S // HD          # 16 new-key chunks
F16 = mybir.dt.float16
F32 = mybir.dt.float32
AF = mybir.ActivationFunctionType
ALU = mybir.AluOpType
INV_SQRT_HD = float(1.0 / np.sqrt(HD))

GROUPS = [[0, 1, 2, 3], [4, 5, 6, 7]]


def build():
    nc = bacc.Bacc("TRN2", target_bir_lowering=False, debug=False, num_devices=8)

    def inp(name, shape, dt=F16):
        return nc.dram_tensor(name, shape, dt, kind="ExternalInput").ap()

    # all host-side pre-packed for contiguous per-partition DMA
    xp = inp("xp", [HD, NDC, S])          # xp[p,kc,s] = x[b][s, kc*128+p]
    wqp = inp("wqp", [HD, GH, NDC, HD])   # [p,j,kc,n] = wq[kc*128+p, 512g+128j+n]
    wkp = inp("wkp", [HD, GH, NDC, HD])
    wvp = inp("wvp", [HD, NDC, DG])       # [p,kc,n] = wv[kc*128+p, sl_n]
    bqp = inp("bqp", [HD, GH], F32)       # [p,j] = bq[sl][128j+p]
    bkp = inp("bkp", [HD, GH], F32)
    bvp = inp("bvp", [1, DG])
    bop = inp("bop", [1, DG])
    ckp = inp("ckp", [HD, GH, PC])        # [p,j,key] = cache_k[b][key, 512g+128j+p]
    cvp = inp("cvp", [HD, NCC, DG])       # [p,ss,d] = cache_v[b][ss*128+p, sl_d]
    wop = inp("wop", [HD, 16, DG])        # rows permuted to gather order, packed
    y = nc.dram_tensor("y", [S, DG], F32, kind="ExternalOutput").ap()

    with tile.TileContext(nc) as tc, ExitStack() as ctx:
        res = ctx.enter_context(tc.tile_pool(name="res", bufs=1))
        dram = ctx.enter_context(tc.tile_pool(name="dram", bufs=1, space="DRAM"))
        ps = ctx.enter_context(tc.tile_pool(name="ps", bufs=1, space="PSUM"))

        # whole-kernel residents
        bq_t = res.tile([HD, GH], F32, tag="bq")
        bk_t = res.tile([HD, GH], F32, tag="bk")
        bv_t = res.tile([1, DG], F16, tag="bv")
        bo_t = res.tile([1, DG], F16, tag="bo")
        ones_k = res.tile([HD, 1], F16, tag="ones_k")      # [128,1] ones
        ones_r16 = res.tile([1, HD], F16, tag="ones_r16")  # [1,128] ones
        ones_r32 = res.tile([1, HD], F32, tag="ones_r32")
        nc.sync.dma_start(bq_t[:], bqp)
        nc.sync.dma_start(bk_t[:], bkp)
        nc.sync.dma_start(bv_t[:], bvp)
        nc.sync.dma_start(bo_t[:], bop)
        nc.vector.memset(ones_k[:], 1.0)
        nc.vector.memset(ones_r16[:], 1.0)
        nc.vector.memset(ones_r32[:], 1.0)

        # broadcast bias tiles (bias varies along the free dim, so build
        # [128, 512] broadcast copies once via ones-matmuls)
        bv_bc = res.tile([HD, DG], F16, tag="bv_bc")
        bo_bc = res.tile([HD, DG], F32, tag="bo_bc")
        psx = ps.tile([HD, DG], F32, tag="pq0", name="ps_bv")
        nc.tensor.matmul(psx[:], ones_r16[:], bv_t[:], start=True, stop=True)
        nc.vector.tensor_copy(bv_bc[:], psx[:])
        psx = ps.tile([HD, DG], F32, tag="PA0", name="ps_bo")
        nc.tensor.matmul(psx[:], ones_r16[:], bo_t[:], start=True, stop=True)
        nc.vector.tensor_copy(bo_bc[:], psx[:])

        # collective bounce buffers: heads 0-2 gather per sb-pair half so
        # each half-gather fires as soon as two sb blocks are normalized;
        # head 3 gathers per sb quarter to minimize the end-of-kernel wait
        bounce_in = {}
        bounce_out = {}
        for j in range(3):
            for h in range(2):
                bounce_in[j, h] = dram.tile([HD, 2, NB], F16,
                                            tag=f"bi{j}_{h}", name=f"bi{j}_{h}")
                bounce_out[j, h] = dram.tile([4, HD, 2, NB], F16,
                                             tag=f"bg{j}_{h}",
                                             name=f"bg{j}_{h}")
        for qq in range(4):
            bounce_in[3, qq] = dram.tile([HD, 1, NB], F16,
                                         tag=f"bi3_{qq}", name=f"bi3_{qq}")
            bounce_out[3, qq] = dram.tile([4, HD, 1, NB], F16,
                                          tag=f"bg3_{qq}", name=f"bg3_{qq}")

        # long-lived attention pools (heads 0..3)
        hp = ctx.enter_context(tc.tile_pool(name="hp", bufs=2))
        vp = ctx.enter_context(tc.tile_pool(name="vp", bufs=1))
        ep = ctx.enter_context(tc.tile_pool(name="ep", bufs=8))
        zp = ctx.enter_context(tc.tile_pool(name="zp", bufs=2))
        apool = ctx.enter_context(tc.tile_pool(name="ap", bufs=2))

        cv_t = vp.tile([HD, NCC, DG], F16, tag="cv")
        vn_t = vp.tile([HD, NSS, DG], F16, tag="vn")

        pq_cnt = [0]
        PRO_TAGS = ["pq0", "PA0", "pss0", "pss1", "pss2"]

        def proj_chunk(wt, kc_tiles, dst_ap, bias_ap, tags=("pq0",)):
            """One [128, 512] projection chunk: 16 accumulating matmuls +
            DVE evacuation with per-partition bias add."""
            psq = ps.tile([HD, NB], F32, tag=tags[pq_cnt[0] % len(tags)],
                          name="psq")
            pq_cnt[0] += 1
            for kc in range(NDC):
                nc.tensor.matmul(psq[:], wt[:, kc, :], kc_tiles[kc],
                                 start=(kc == 0), stop=(kc == NDC - 1))
            nc.vector.tensor_scalar(dst_ap, psq[:], bias_ap, None, ALU.add)

        def v_chunk(ss, xg):
            psv = ps.tile([HD, DG], F32,
                          tag=PRO_TAGS[pq_cnt[0] % len(PRO_TAGS)], name="psv")
            pq_cnt[0] += 1
            for kc in range(NDC):
                nc.tensor.matmul(psv[:],
                                 xg[kc // 4][:, kc % 4, HD * ss:HD * (ss + 1)],
                                 wvt[:, kc, :],
                                 start=(kc == 0), stop=(kc == NDC - 1))
            # vn = psv + bv (bias varies along free dim -> broadcast add)
            nc.vector.scalar_tensor_tensor(vn_t[:, ss, :], psv[:], 1.0,
                                           bv_bc[:], ALU.mult, ALU.add)

        def attention(j, qT, kT, ckT, quarters=False):
            nah = 4 if quarters else 2
            aheads = [apool.tile([HD, 2 // (nah // 2), NB], F16, tag=f"ah{h}",
                                 name=f"ah{j}_{h}")
                      for h in range(nah)]
            for sb in range(4):
                PA = ps.tile([HD, NB], F32, tag="PA0", name="PA")
                zacc = zp.tile([HD, 2, NB], F16, tag="z")
                qs = qT[:, NB * sb:NB * (sb + 1)]
                for c2 in range(NKC // 2):
                    pss = ps.tile([HD, 2, NB], F32, tag=f"pss{c2 % 3}",
                                  name="pss")
                    e2 = ep.tile([HD, 2, NB], F16, tag="e")
                    for i in range(2):
                        c = 2 * c2 + i
                        if c < NCC:
                            kt = ckT[:, HD * c:HD * (c + 1)]
                        else:
                            kt = kT[:, HD * (c - NCC):HD * (c - NCC + 1)]
                        nc.tensor.matmul(pss[:, i, :], kt, qs,
                                         start=True, stop=True)
                    nc.scalar.activation(e2[:], pss[:], AF.Exp,
                                         scale=INV_SQRT_HD)
                    if c2 == 0:
                        nc.vector.tensor_copy(zacc[:], e2[:])
                    else:
                        nc.vector.tensor_tensor(zacc[:], zacc[:], e2[:],
                                                ALU.add)
                    for i in range(2):
                        c = 2 * c2 + i
                        if c < NCC:
                            vt = cv_t[:, c, HD * j:HD * (j + 1)]
                        else:
                            vt = vn_t[:, c - NCC, HD * j:HD * (j + 1)]
                        nc.tensor.matmul(PA[:], vt, e2[:, i, :],
                                         start=(c == 0), stop=(c == NKC - 1),
                                         skip_group_check=True)
                # softmax denominator: accumulate the two zacc halves via two
                # ones-matmuls, fast reciprocal, broadcast, normalize PA
                psz = ps.tile([1, NB], F32, tag="pss1", name="psz")
                nc.tensor.matmul(psz[:], ones_k[:], zacc[:, 0, :],
                                 start=True, stop=False)
                nc.tensor.matmul(psz[:], ones_k[:], zacc[:, 1, :],
                                 start=False, stop=True)
                zinv = zp.tile([1, NB], F32, tag="zi")
                nc.vector.reciprocal_approx_fast(zinv[:], psz[:])
                zinv16 = zp.tile([1, NB], F16, tag="zi16")
                nc.vector.tensor_copy(zinv16[:], zinv[:])
                psb = ps.tile([HD, NB], F32, tag="pss2", name="psb")
                nc.tensor.matmul(psb[:], ones_r16[:], zinv16[:],
                                 start=True, stop=True)
                zb = zp.tile([HD, NB], F32, tag="zb")
                nc.vector.tensor_copy(zb[:], psb[:])
                if quarters:
                    nc.vector.tensor_tensor(aheads[sb][:, 0, :], PA[:],
                                            zb[:], ALU.mult)
                    nc.sync.dma_start(bounce_in[j, sb][:], aheads[sb][:])
                    nc.gpsimd.collective_compute(
                        "AllGather", ALU.bypass, replica_groups=GROUPS,
                        ins=[bounce_in[j, sb].opt()],
                        outs=[bounce_out[j, sb].opt()])
                else:
                    nc.vector.tensor_tensor(aheads[sb // 2][:, sb % 2, :],
                                            PA[:], zb[:], ALU.mult)
                    if sb % 2 == 1:
                        h = sb // 2
                        nc.sync.dma_start(bounce_in[j, h][:], aheads[h][:])
                        nc.gpsimd.collective_compute(
                            "AllGather", ALU.bypass, replica_groups=GROUPS,
                            ins=[bounce_in[j, h].opt()],
                            outs=[bounce_out[j, h].opt()])

        with ExitStack() as xw:
            xpool = xw.enter_context(tc.tile_pool(name="xp", bufs=1))
            wpool = xw.enter_context(tc.tile_pool(name="wp", bufs=2))
            vwpool = xw.enter_context(tc.tile_pool(name="vw", bufs=1))

            # weights for head 0 first (q0's first matmul needs them), then x
            # as a serialized stream so the first groups land ASAP; the bulk
            # loads (wv, cache) are forced behind x so they don't steal HBM
            # bandwidth from the critical startup path
            wq_t = {0: wpool.tile([HD, NDC, HD], F16, tag="wq", name="wq0")}
            wk_t = {0: wpool.tile([HD, NDC, HD], F16, tag="wk", name="wk0")}
            nc.sync.dma_start(wq_t[0][:], wqp[:, 0])
            nc.sync.dma_start(wk_t[0][:], wkp[:, 0])
            xg = []
            last_xdma = None
            for i in range(4):
                t = xpool.tile([HD, 4, S], F16, tag=f"xg{i}", name=f"xg{i}")
                last_xdma = nc.sync.dma_start(t[:], xp[:, 4 * i:4 * (i + 1), :])
                xg.append(t)
            # wv/cache-v are not needed until the v pass / PV chains; issue
            # them from the gpsimd software DGE gated on the x stream so the
            # HW queues give x the full HBM bandwidth at startup
            wvt = vwpool.tile([HD, NDC, DG], F16, tag="wvt")
            for dst, src in ((wvt[:], wvp), (cv_t[:], cvp)):
                di = nc.gpsimd.dma_start(dst, src)
                tile.add_dep_helper(di.ins, last_xdma.ins,
                                    reason="bulk loads after x stream")
            ck_t = {0: hp.tile([HD, PC], F16, tag="ckT", name="ck0")}
            di = nc.gpsimd.dma_start(ck_t[0][:], ckp[:, 0, :])
            tile.add_dep_helper(di.ins, last_xdma.ins,
                                reason="cache-k after x stream")

            def proj_head(j):
                qT = hp.tile([HD, S], F16, tag="qT", name=f"qT{j}")
                kT = hp.tile([HD, S], F16, tag="kT", name=f"kT{j}")
                for sb in range(4):
                    kcs = [xg[kc // 4][:, kc % 4, NB * sb:NB * (sb + 1)]
                           for kc in range(NDC)]
                    proj_chunk(wq_t[j][:], kcs, qT[:, NB * sb:NB * (sb + 1)],
                               bq_t[:, j:j + 1])
                    proj_chunk(wk_t[j][:], kcs, kT[:, NB * sb:NB * (sb + 1)],
                               bk_t[:, j:j + 1])
                return qT, kT

            def proj_head0():
                """Head-0 q/k with kc-group-major order over 6 concurrent
                PSUM chains, so matmuls track the streaming x DMA instead of
                stalling a single chain on each x group."""
                qT = hp.tile([HD, S], F16, tag="qT", name="qT0")
                kT = hp.tile([HD, S], F16, tag="kT", name="kT0")
                chains = [("q", sb) for sb in range(4)] + [("k", 0)]
                psqs = [ps.tile([HD, NB], F32, tag=PRO_TAGS[ci],
                                name=f"ps0_{ci}")
                        for ci in range(len(chains))]
                for grp in range(4):
                    for ci, (w, sb) in enumerate(chains):
                        wt = wq_t[0] if w == "q" else wk_t[0]
                        for kc in range(4 * grp, 4 * grp + 4):
                            nc.tensor.matmul(
                                psqs[ci][:], wt[:, kc, :],
                                xg[grp][:, kc % 4, NB * sb:NB * (sb + 1)],
                                start=(kc == 0), stop=(kc == NDC - 1))
                for ci, (w, sb) in enumerate(chains):
                    dst = qT if w == "q" else kT
                    bias = bq_t[:, 0:1] if w == "q" else bk_t[:, 0:1]
                    nc.vector.tensor_scalar(dst[:, NB * sb:NB * (sb + 1)],
                                            psqs[ci][:], bias, None, ALU.add)
                for sb in (1, 2, 3):
                    kcs = [xg[kc // 4][:, kc % 4, NB * sb:NB * (sb + 1)]
                           for kc in range(NDC)]
                    proj_chunk(wk_t[0][:], kcs, kT[:, NB * sb:NB * (sb + 1)],
                               bk_t[:, 0:1], tags=PRO_TAGS)
                return qT, kT

            def prefetch_head(jn):
                wq_t[jn] = wpool.tile([HD, NDC, HD], F16, tag="wq",
                                      name=f"wq{jn}")
                wk_t[jn] = wpool.tile([HD, NDC, HD], F16, tag="wk",
                                      name=f"wk{jn}")
                nc.sync.dma_start(wq_t[jn][:], wqp[:, jn])
                nc.sync.dma_start(wk_t[jn][:], wkp[:, jn])
                ck_t[jn] = hp.tile([HD, PC], F16, tag="ckT", name=f"ck{jn}")
                nc.sync.dma_start(ck_t[jn][:], ckp[:, jn, :])

            # head 0 projections, then the full v pass
            with nc.named_scope("proj0"):
                qkT = {0: proj_head0()}
                prefetch_head(1)
            with nc.named_scope("vpass"):
                for ss in range(NSS):
                    v_chunk(ss, xg)

            # heads 0-1: attention + later heads' projections as PE filler;
            # head-3's projection runs in head 1 so x/weight space frees
            # early enough for the out-proj tiles to load during head 2
            with nc.named_scope("head0"):
                attention(0, qkT[0][0], qkT[0][1], ck_t[0][:])
                qkT[1] = proj_head(1)
                prefetch_head(2)
            with nc.named_scope("head1"):
                attention(1, qkT[1][0], qkT[1][1], ck_t[1][:])
                qkT[2] = proj_head(2)
                prefetch_head(3)
                qkT[3] = proj_head(3)

        # phase 3 pools (reuse the x/weight SBUF space released above)
        with tc.tile_pool(name="wo3", bufs=1) as wop_pool, \
             tc.tile_pool(name="lt3", bufs=1) as ltp, \
             tc.tile_pool(name="y12", bufs=1) as y12p, \
             tc.tile_pool(name="yo", bufs=3) as yop:
            wot = wop_pool.tile([HD, 16, NB], F16, tag="wo")
            nc.sync.dma_start(wot[:], wop)
            y12 = y12p.tile([HD, NSS, NB], F16, tag="y12")
            lts = {}

            def load_lts(j):
                nh = 4 if j == 3 else 2
                for h in range(nh):
                    for r in range(4):
                        jr = 4 * j + r
                        lt = ltp.tile([HD, 2 // (nh // 2), NB], F16,
                                      tag=f"lt{jr}_{h}", name=f"lt{jr}_{h}")
                        nc.sync.dma_start(lt[:], bounce_out[j, h][r])
                        lts[jr, h] = lt

            oc = [0]

            def oproj_chunk(m, js, mode):
                """One [128, 512] out-proj chunk accumulating the listed
                heads' 4-peer contributions. Filler passes ('first'/'acc')
                stay on pq0 so they are usable as attention filler; the
                final pass alternates pq0/PA0 for a dense tail."""
                if mode == "final":
                    tag = ("pq0", "PA0")[oc[0] % 2]
                    oc[0] += 1
                else:
                    tag = "pq0"
                psO = ps.tile([HD, NB], F32, tag=tag, name=f"psO{js[0]}")
                mms = [(j, 4 * j + r) for j in js for r in range(4)]
                for n, (j, jr) in enumerate(mms):
                    if j == 3:
                        lt = lts[jr, m // 4][:, 0, HD * (m % 4):HD * (m % 4 + 1)]
                    else:
                        lt = lts[jr, m // 8][:, (m // 4) % 2,
                                             HD * (m % 4):HD * (m % 4 + 1)]
                    nc.tensor.matmul(
                        psO[:], lt, wot[:, jr, :],
                        start=(n == 0), stop=(n == len(mms) - 1),
                        skip_group_check=True)
                if mode == "first":
                    nc.vector.scalar_tensor_tensor(
                        y12[:, m, :], psO[:], 1.0, bo_bc[:],
                        ALU.mult, ALU.add)
                elif mode == "acc":
                    nc.vector.tensor_tensor(y12[:, m, :], y12[:, m, :],
                                            psO[:], ALU.add)
                else:
                    ot = yop.tile([HD, NB], F32, tag="ot")
                    nc.vector.tensor_tensor(ot[:], psO[:], y12[:, m, :],
                                            ALU.add)
                    nc.sync.dma_start(y[HD * m:HD * (m + 1), :], ot[:])

            load_lts(0)
            load_lts(1)
            # head 2: attention with head-0's out-proj partial as PE filler
            with nc.named_scope("head2"):
                attention(2, qkT[2][0], qkT[2][1], ck_t[2][:])
            with nc.named_scope("oproj0"):
                for m in range(NSS):
                    oproj_chunk(m, [0], "first")
            load_lts(2)
            # head 3: attention with heads-1/2 out-proj partials as PE filler
            with nc.named_scope("head3"):
                attention(3, qkT[3][0], qkT[3][1], ck_t[3][:], quarters=True)
            with nc.named_scope("oproj1"):
                for m in range(NSS):
                    oproj_chunk(m, [1], "acc")
            load_lts(3)
            # final: heads-2/3 contributions + combine + store; the first
            # half of the sequence only needs the first half-gathers
            with nc.named_scope("oproj3"):
                for m in range(NSS):
                    oproj_chunk(m, [2, 3], "final")

    nc.compile()
    return nc


_BUILT = None


def get_built():
    global _BUILT
    if _BUILT is None:
        _BUILT = build()
    return _BUILT


def _pack_kc(a):
    """[D, N] -> [128, D//128, N] with [p, kc, n] = a[kc*128+p, n]."""
    d, n = a.shape
    return np.ascontiguousarray(a.reshape(d // HD, HD, n).transpose(1, 0, 2))


def make_in_maps(x, cache_k, cache_v, wq, bq, wk, bk, wv, bv, wo, bo):
    x = np.asarray(x, np.float32)
    cache_k = np.asarray(cache_k, np.float32)
    cache_v = np.asarray(cache_v, np.float32)
    wq, bq = np.asarray(wq, np.float32), np.asarray(bq, np.float32)
    wk, bk = np.asarray(wk, np.float32), np.asarray(bk, np.float32)
    wv, bv = np.asarray(wv, np.float32), np.asarray(bv, np.float32)
    wo, bo = np.asarray(wo, np.float32), np.asarray(bo, np.float32)

    # permute wo rows to match gather order: chunk jr=(4j+r) holds head 4r+j
    perm = np.concatenate([
        np.arange(HD * (4 * r + j), HD * (4 * r + j) + HD)
        for j in range(GH) for r in range(4)
    ])
    wo_p = wo[perm, :]

    # per-batch packed x: [128, 16, S]
    xp_b = [_pack_kc(np.ascontiguousarray(x[b].T)).astype(np.float16)
            for b in range(B)]
    # per-batch packed cache_v rows: [128, 16, D] then slice cols per core
    cv_b = [np.ascontiguousarray(
        cache_v[b].reshape(NCC, HD, D).transpose(1, 0, 2)) for b in range(B)]

    in_maps = []
    for c in range(8):
        b, g = divmod(c, 4)
        sl = slice(DG * g, DG * (g + 1))
        wq_s, wk_s = wq[:, sl], wk[:, sl]
        # [p, j, kc, n]
        wqp = np.ascontiguousarray(
            wq_s.reshape(NDC, HD, GH, HD).transpose(1, 2, 0, 3)
        ).astype(np.float16)
        wkp = np.ascontiguousarray(
            wk_s.reshape(NDC, HD, GH, HD).transpose(1, 2, 0, 3)
        ).astype(np.float16)
        # ckp [p, j, key] = cache_k[b][key, 512g+128j+p]
        ck_s = cache_k[b][:, sl]                      # [PC, 512]
        ckp = np.ascontiguousarray(
            ck_s.reshape(PC, GH, HD).transpose(2, 1, 0)).astype(np.float16)
        in_maps.append({
            "xp": xp_b[b],
            "wqp": wqp,
            "wkp": wkp,
            "wvp": _pack_kc(wv[:, sl]).astype(np.float16),
            "bqp": np.ascontiguousarray(
                bq[sl].reshape(GH, HD).T).astype(np.float32),
            "bkp": np.ascontiguousarray(
                bk[sl].reshape(GH, HD).T).astype(np.float32),
            "bvp": bv[sl][None, :].astype(np.float16),
            "bop": bo[sl][None, :].astype(np.float16),
            "ckp": ckp,
            "cvp": np.ascontiguousarray(cv_b[b][:, :, sl]).astype(np.float16),
            "wop": _pack_kc(wo_p[:, sl]).astype(np.float16),
        })
    return in_maps


def assemble(results):
    out = np.empty((B, S, D), np.float32)
    for c in range(8):
        b, g = divmod(c, 4)
        out[b, :, DG * g:DG * (g + 1)] = results[c]["y"]
    return out


def kernel(**inputs):
    nc = get_built()
    in_maps = make_in_maps(**inputs)
    res = run_bass_kernel_spmd(nc, in_maps, core_ids=list(range(8)))
    return assemble(res.results)
